# revision 87
# baseline (speedup 1.0000x reference)
"""Bayesian curve filter kernel for Trainium2 (8 NeuronCores, SPMD).

Sharding: data-parallel over the 1024 Monte-Carlo samples -> 128 per core
(exactly the SBUF partition count; samples live on partitions).

Iterated 265us (v1 two-level soft-select over all 1000 boundary points)
-> 95us (v2: 128-pt subsample, select-direct) -> 33us (final). Final
design:
  * Boundary sets are SUBSAMPLED host-side to 64*nch points per boundary.
    nch is chosen adaptively: the host replays the full reference pipeline
    in fp64 twice -- once with exact nearest-neighbor distances, once
    simulating the device's soft-select math (exp weights, bf16 underflow
    flush) on the subsampled set -- and accepts the smallest nch whose
    final-output deviation is < 1e-3 relative (64/boundary gives 2.5e-6
    on the track data; worst case falls back to the full set).
  * No coarse/max pass at all: the per-(s,p) score shift m is replaced by
    the analytic bound mb_bd(|p|) = 2*max|b|*|p| - min(|b|^2+Csh), folded
    into the pg input grid and embedded per-boundary via indicator rows
    in the score matmul. Any constant column shift cancels in the
    normalized select ratio; only exp over/underflow range matters, which
    the host verifies (K is capped by the measured mb-to-max gap).
  * Both boundaries share one 128-row chunk (64 points each). One score
    matmul per quad produces t2 = mb - s1 for both boundaries; one ACT exp
    per quad-PAIR ([128,1024]) gives the ~one-hot H. The exp stream is the
    pacer; everything else hides behind it.
  * "Select-direct": H column-slices are used as matmul WEIGHTS
    (lhsT = Ht[128b, 128s], rhs = table[128b, 16v] with boundary-masked
    column halves), so the select lands directly in [sample, var] layout
    in per-chunk PSUM tiles -- no transposes, no PSUM->SBUF copies, and
    no write-after-read coupling between select and distance phases.
  * Distance/max phase runs incrementally per 4-quad chunk on Vector,
    overlapped with the PE/ACT loop; per-chunk maxes land in separate
    dmacc columns (no serial accumulation chain).
  * sqrt is table-free (no ACT table switches, exps ungated): speed via
    max+0.5*min chord init (always within 11.8%) + 2 Newton steps;
    centripetal max via a host-fitted chord init clipped at the
    don't-care threshold + 3 Newton steps. Braking interp collapses to a
    2-op clip when the (float32) table is linear, verified by endpoint
    deviation. The boundary exp folds into the score exp when the host
    proves the reference's 1e-32 clamp inactive.

Device algorithm per core:
  1. out1 = curves^T @ R : per-sample curve points / velocity / accel.
  2. speeds / centripetal / braking pipeline on [128, 60] tiles (filler).
  3. Per 512-col quad (4 p's x 128 samples), per chunk:
       t2[b,(p,s)] = mb - s1   (one [8,128]x[8,512] matmul; rows
                                px,py,1,1,mb0hi,mb0lo,mb1hi,mb1lo)
       H = exp(-K t2 - 25)     (one ACT instruction)
       sel[s,16v] = H-slice^T @ tbl   (4 select-direct matmuls)
     then dist = (Se - px*Scx - py*Scy)/Sn and a running max over (p,bd).
  4. Per-sample log-score -> w; partial (sum_s w*curve_s, sum_s w) via a
     final [128,17]x[128,1] matmul -> [17] per core; host sums across the
     8 cores and divides (softmax normalization cancels globally).
"""

import numpy as np

import concourse.bass as bass
import concourse.bacc as bacc
import concourse.mybir as mybir
from concourse import tile
from concourse import bass_utils

F32 = mybir.dt.float32
BF16 = mybir.dt.bfloat16
F16 = mybir.dt.float16
ALU = mybir.AluOpType
AF = mybir.ActivationFunctionType
AX = mybir.AxisListType

NCORES = 8
S_FULL = 1024
SC = 128          # samples per core
P = 60            # points per curve
ORD = 7           # bezier order
BETA_SPEED = 0.1
MAX_CA = 19.6
NSEG = 19         # interp segments (20 knots)
NQ = 15           # p-quads (4 p's x 128 samples = 512 cols each)

_cache = {}


def _diff_mat(n):
    # D [n, n+1]: (D @ c)[k] = c[k+1] - c[k]
    D = np.zeros((n, n + 1), np.float64)
    for k in range(n):
        D[k, k] = -1.0
        D[k, k + 1] = 1.0
    return D


def _build_program(interp, nch, sq, fuse_w, split_h):
    """interp = (lin, xs, dxs, ms, y0, lo, hi); nch chunks of 128 boundary
    rows (64 per bd); sq = ('newton', (m,M)_spd, (m,M)_cam) or ('act',);
    fuse_w folds the boundary exp into the score exp (clamp inactive).
    The pg grid (points + mb rows) is a host input."""
    lin, interp_x, interp_dx, interp_m, y0, blo, bhi = interp
    nc = bacc.Bacc("TRN2", target_bir_lowering=False, debug=False, enable_asserts=False)

    # ---- DRAM I/O ----
    F32R = mybir.dt.float32r
    d_cv = nc.dram_tensor("cv", [16, SC], F32R, kind="ExternalInput").ap()      # curvesT: rows 0-7 x-coefs, 8-15 y
    d_cf = nc.dram_tensor("cf17", [SC, 17], F32, kind="ExternalInput").ap()     # curves flat + ones col
    d_R = nc.dram_tensor("Rm", [8, 180], F32R, kind="ExternalInput").ap()
    d_bG = nc.dram_tensor("bG", [8, 128 * nch], F16, kind="ExternalInput").ap() # [-2bx;-2by;b2Chi;b2Clo;I0;I0;I1;I1]
    d_tb = nc.dram_tensor("tb", [SC, 16 * nch], BF16, kind="ExternalInput").ap()  # bd-masked select tables
    d_Kv = nc.dram_tensor("Kv", [SC, 1], F32, kind="ExternalInput").ap()        # -K replicated
    d_Th = nc.dram_tensor("Th", [SC, 1], F32, kind="ExternalInput").ap()        # is_le threshold
    d_pg = nc.dram_tensor("pgt", [8, P * SC], F16, kind="ExternalInput").ap()   # [px;py;1;1;mb0hi;mb0lo;mb1hi;mb1lo]
    d_out = nc.dram_tensor("out17", [17, 1], F32, kind="ExternalOutput").ap()

    with tile.TileContext(nc) as tc:
        with (
            tc.tile_pool(name="cst", bufs=1) as cst,
            tc.tile_pool(name="hbuf", bufs=nch + 3) as hbuf,
            tc.tile_pool(name="wk", bufs=4) as wk,
            tc.tile_pool(name="pt2", bufs=2, space="PSUM") as pt2,    # [128,1024] t2 / startup matmuls
            tc.tile_pool(name="pdt", bufs=4, space="PSUM") as pdt,    # per-pair select outputs
        ):
            # ---- load constants; in amb mode the exps are ungated so
            # pg/bGs/Kv (-> first NN -> exp) go first; in hybrid/act the
            # cvx/Rm -> o1x -> spd chain gates the exps and loads first ----
            pg = cst.tile([8, P * SC], F16)
            cvx = cst.tile([8, SC], F32R)
            Rm = cst.tile([8, 180], F32R)
            bGs = cst.tile([8, 128 * nch], F16)
            Kv = cst.tile([SC, 1], F32)
            # cvx/Rm first: o1x sits at the head of the PE queue, so its
            # inputs must arrive before pg to avoid stalling the first NN.
            nc.sync.dma_start(cvx[:], d_cv[0:8, :])
            nc.sync.dma_start(Rm[:], d_R)
            nc.sync.dma_start(pg[0:3, :], d_pg[0:3, :])
            nc.scalar.dma_start(bGs[:], d_bG)
            nc.scalar.dma_start(Kv[:], d_Kv)
            nc.scalar.dma_start(pg[6:8, :], d_pg[6:8, :])
            cvy = cst.tile([8, SC], F32R)
            nc.gpsimd.dma_start(cvy[:], d_cv[8:16, :])
            nc.gpsimd.dma_start(pg[3:6, :], d_pg[3:6, :])
            tbm = cst.tile([SC, 16 * nch], BF16)
            nc.scalar.dma_start(tbm[:], d_tb)
            cf = cst.tile([SC, 17], F32)
            nc.scalar.dma_start(cf[:], d_cf)
            thv = cst.tile([SC, 1], F32)
            nc.scalar.dma_start(thv[:], d_Th)

            # ---- pts/vel/accel in [s, col] layout ----
            o1x = pt2.tile([SC, 180], F32, tag="t2")
            nc.tensor.matmul(o1x[:], cvx[:], Rm[:], start=True, stop=True)
            o1y = pt2.tile([SC, 180], F32, tag="t2")
            nc.tensor.matmul(o1y[:], cvy[:], Rm[:], start=True, stop=True)
            ox = cst.tile([SC, 180], F32)
            nc.vector.tensor_copy(ox[:], o1x[:])
            oy = cst.tile([SC, 180], F32)
            nc.vector.tensor_copy(oy[:], o1y[:])
            # phase-C coefficient grid: pxy3[s, (p, bd, 3)] = (1, -px, -py)
            pxy3 = cst.tile([SC, 6 * P], F32)
            nc.vector.memset(pxy3[:], 1.0)
            nc.vector.tensor_scalar(
                pxy3[:, 1:6 * P:3].rearrange("s (p b) -> s p b", b=2),
                ox[:, 0:P].rearrange("s (p b) -> s p b", b=1).to_broadcast((SC, P, 2)),
                -1.0, 0.0, op0=ALU.mult, op1=ALU.add)
            nc.vector.tensor_scalar(
                pxy3[:, 2:6 * P:3].rearrange("s (p b) -> s p b", b=2),
                oy[:, 0:P].rearrange("s (p b) -> s p b", b=1).to_broadcast((SC, P, 2)),
                -1.0, 0.0, op0=ALU.mult, op1=ALU.add)

            # ---- dedicated tiles for the speeds/interp filler pipeline ----
            vx, vy, ax_, ay = (ox[:, 60:120], oy[:, 60:120], ox[:, 120:180], oy[:, 120:180])
            spd2 = cst.tile([SC, P], F32)
            t0 = cst.tile([SC, P], F32)
            spd = cst.tile([SC, P], F32)
            rspd = cst.tile([SC, P], F32)
            adv = cst.tile([SC, P], F32)
            lin_ = cst.tile([SC, P], F32)
            a2 = cst.tile([SC, P], F32)
            camax2 = cst.tile([SC, 1], F32)
            camax = cst.tile([SC, 1], F32)
            avg = cst.tile([SC, 1], F32)
            bl = cst.tile([SC, P], F32)
            ti = cst.tile([SC, P], F32)
            bv = cst.tile([SC, P], F32)
            worst = cst.tile([SC, 1], F32)

            # ---- speeds/accel/braking pipeline (DAG-scheduled as filler) ----
            def newton_sqrt(y, x, rng, n_iter, tiles):
                # y = sqrt(x) via Newton from a chord init (exact at m and M);
                # iterates approach from above, table-free.
                m, M = rng
                G = float(np.sqrt(m * M))
                Sv = float(1.0 / (np.sqrt(m) + np.sqrt(M)))
                nc.vector.tensor_scalar(y[:], x, Sv, G * Sv, op0=ALU.mult, op1=ALU.add)
                rr, tt = tiles
                for _ in range(n_iter):
                    nc.vector.reciprocal(rr[:], y[:])
                    nc.vector.tensor_mul(tt[:], x, rr[:])
                    nc.vector.tensor_add(tt[:], tt[:], y[:])
                    nc.vector.tensor_scalar(y[:], tt[:], 0.5, 0.0, op0=ALU.mult, op1=ALU.add)

            nc.vector.tensor_mul(spd2[:], vx, vx)
            nc.vector.tensor_mul(t0[:], vy, vy)
            nc.vector.tensor_add(spd2[:], spd2[:], t0[:])
            if sq[0] == "amb":
                # range-free: y0 = max(|vx|,|vy|) + 0.5*min in [1, 1.118]*|v|,
                # then 3 table-free Newton steps (converging from above).
                axv = cst.tile([SC, P], F32)
                nc.vector.tensor_scalar(axv[:], vx, -1.0, 0.0, op0=ALU.mult, op1=ALU.add)
                nc.vector.tensor_max(axv[:], axv[:], vx)                  # |vx|
                ayv = cst.tile([SC, P], F32)
                nc.vector.tensor_scalar(ayv[:], vy, -1.0, 0.0, op0=ALU.mult, op1=ALU.add)
                nc.vector.tensor_max(ayv[:], ayv[:], vy)                  # |vy|
                nwr = cst.tile([SC, P], F32)
                nwt = cst.tile([SC, P], F32)
                nc.vector.tensor_add(nwt[:], axv[:], ayv[:])              # |vx|+|vy|
                nc.vector.tensor_max(axv[:], axv[:], ayv[:])
                nc.vector.tensor_sub(nwt[:], nwt[:], axv[:])              # min(|vx|,|vy|)
                nc.vector.tensor_scalar(nwt[:], nwt[:], 0.5, 0.0, op0=ALU.mult, op1=ALU.add)
                nc.vector.tensor_add(spd[:], axv[:], nwt[:])
                for _ in range(2):
                    nc.vector.reciprocal(nwr[:], spd[:])
                    nc.vector.tensor_mul(nwt[:], spd2[:], nwr[:])
                    nc.vector.tensor_add(nwt[:], nwt[:], spd[:])
                    nc.vector.tensor_scalar(spd[:], nwt[:], 0.5, 0.0, op0=ALU.mult, op1=ALU.add)
            elif sq[0] == "newton":
                nwr = cst.tile([SC, P], F32)
                nwt = cst.tile([SC, P], F32)
                newton_sqrt(spd, spd2[:], sq[1], 3, (nwr, nwt))
            else:
                nc.scalar.activation(spd[:], spd2[:], AF.Sqrt)
            nc.vector.reciprocal(rspd[:], spd[:])
            nc.vector.tensor_mul(adv[:], ax_, vx)
            nc.vector.tensor_mul(t0[:], ay, vy)
            nc.vector.tensor_add(adv[:], adv[:], t0[:])
            nc.vector.tensor_mul(lin_[:], adv[:], rspd[:])
            nc.vector.tensor_mul(a2[:], ax_, ax_)
            nc.vector.tensor_mul(t0[:], ay, ay)
            nc.vector.tensor_add(a2[:], a2[:], t0[:])
            nc.vector.tensor_mul(t0[:], lin_[:], lin_[:])
            nc.vector.tensor_sub(a2[:], a2[:], t0[:])  # ca^2 (may be ~-eps)
            nc.vector.tensor_reduce(camax2[:], a2[:], axis=AX.X, op=ALU.max)
            if sq[0] in ("newton", "hybrid", "amb"):
                # clip below the don't-care threshold; Newton stays >= sqrt
                rng_c = sq[2] if sq[0] == "newton" else sq[1]
                nc.vector.tensor_scalar_max(camax2[:], camax2[:], float(rng_c[0]))
                nwr1 = cst.tile([SC, 1], F32)
                nwt1 = cst.tile([SC, 1], F32)
                newton_sqrt(camax, camax2[:], rng_c, 3, (nwr1, nwt1))
            else:
                nc.vector.tensor_scalar_max(camax2[:], camax2[:], 0.0)
                nc.scalar.activation(camax[:], camax2[:], AF.Sqrt)
            nc.vector.tensor_reduce(avg[:], spd[:], axis=AX.X, op=ALU.add)
            if lin:
                # braking table is linear: bl = clip(m*spd + a, lo, hi)
                a0 = float(y0 - interp_m[0] * interp_x[0])
                nc.vector.tensor_scalar(bl[:], spd[:], float(interp_m[0]), a0,
                                        op0=ALU.mult, op1=ALU.add)
                nc.vector.tensor_scalar(bl[:], bl[:], float(blo), float(bhi),
                                        op0=ALU.max, op1=ALU.min)
            else:
                nc.vector.memset(bl[:], float(y0))
                for i in range(NSEG):
                    nc.vector.tensor_scalar(ti[:], spd[:], float(interp_x[i]), 0.0,
                                            op0=ALU.subtract, op1=ALU.max)
                    nc.vector.tensor_scalar(ti[:], ti[:], float(interp_dx[i]), float(interp_m[i]),
                                            op0=ALU.min, op1=ALU.mult)
                    nc.vector.tensor_add(bl[:], bl[:], ti[:])
            nc.vector.tensor_sub(bv[:], lin_[:], bl[:])
            nc.vector.tensor_reduce(worst[:], bv[:], axis=AX.X, op=ALU.min)
            nc.vector.tensor_scalar_min(worst[:], worst[:], 0.0)

            b25 = cst.tile([SC, 1], F32)
            if sq[0] in ("newton", "amb"):
                # no ACT table switches at all: exps are ungated
                nc.vector.memset(b25[:], -25.0)
            elif sq[0] == "hybrid":
                # gate exps on the (only) ACT sqrt: one table switch, early
                nc.vector.tensor_scalar(b25[:], spd[:, 0:1], 0.0, -25.0, op0=ALU.mult, op1=ALU.add)
            else:
                # b25 depends on camax so both Sqrt activations are forced
                # before the first Exp -- one ACT table switch each way.
                nc.vector.tensor_scalar(b25[:], camax[:], 0.0, -25.0, op0=ALU.mult, op1=ALU.add)

            # ---- per-pair boundary pipeline (2 quads = 8 p's per step) ----
            NP2 = (NQ + 1) // 2   # 8 pairs (last pair holds one quad)
            # distance-phase chunks (first-quad, n-quads, first-pair, last-pair):
            # trailing chunks shrink so the post-loop chain is one quad wide
            CHK = [(0, 4, 0, 1), (4, 4, 2, 3), (8, 4, 4, 5),
                   (12, 2, 6, 6), (14, 1, 7, 7)]
            chunk_of_pair = {}
            for j, (q0, nqj, p0, p1) in enumerate(CHK):
                for k in range(p0, p1 + 1):
                    chunk_of_pair[k] = (j, 2 * k - q0)   # quad offset in chunk
            dmacc = cst.tile([SC, len(CHK)], F32)
            hts = {}
            dTp = {}

            def em2a(k):
                nq = 2 if k < NP2 - 1 or NQ % 2 == 0 else 1
                for c in range(nch):
                    t2 = pt2.tile([SC, 512 * nq], F32, tag="t2", name=f"t2_{k}_{c}")
                    for h in range(nq):
                        nc.tensor.matmul(
                            t2[:, h * 512:(h + 1) * 512],
                            bGs[:, c * 128:(c + 1) * 128],
                            pg[:, (2 * k + h) * 512:(2 * k + h + 1) * 512],
                            start=True, stop=True)
                    Ht = hbuf.tile([SC, 512 * nq], BF16, tag="h", name=f"ht_{k}_{c}")
                    if split_h and nq == 2:
                        # ACT and DVE each build half the one-hot weights
                        nc.scalar.activation(Ht[:, 0:512], t2[:, 0:512], AF.Exp,
                                             scale=Kv[:], bias=b25[:])
                        nc.vector.tensor_scalar(Ht[:, 512:1024], t2[:, 512:1024],
                                                thv[:], None, op0=ALU.is_le)
                    else:
                        nc.scalar.activation(Ht[:], t2[:], AF.Exp, scale=Kv[:], bias=b25[:])
                    hts[(k, c)] = Ht

            def em2b(k):
                nq = 2 if k < NP2 - 1 or NQ % 2 == 0 else 1
                j, qoff = chunk_of_pair[k]
                q0, nqj, p0, _ = CHK[j]
                if k == p0:
                    dTp[j] = pdt.tile([SC, 64 * nqj], F32, tag="dt", name=f"dTp{j}")
                off = qoff * 64
                for h in range(nq):
                    for j4 in range(4):
                        o = dTp[j][:, off + h * 64 + j4 * 16: off + h * 64 + (j4 + 1) * 16]
                        for c in range(nch):
                            nc.tensor.matmul(
                                o, hts[(k, c)][:, h * 512 + j4 * 128: h * 512 + (j4 + 1) * 128],
                                tbm[:, c * 16:(c + 1) * 16],
                                start=(c == 0), stop=(c == nch - 1))
                for c in range(nch):
                    del hts[(k, c)]

            def phaseC(j):
                # quad chunk j -> max signed distance into dmacc column j
                q0, nqj, _, _ = CHK[j]
                W = 64 * nqj
                n8 = 8 * nqj   # (4*nqj p's) x 2 bds
                dq = wk.tile([SC, W], F32, tag="pc")
                nc.vector.tensor_copy(dq[:], dTp.pop(j)[:, 0:W])
                dqv = dq[:].rearrange("s (b v) -> s b v", v=8)
                out3 = wk.tile([SC, 3 * n8], F32, tag="se")
                o3v = out3[:].rearrange("s (b v) -> s b v", v=3)
                nc.vector.tensor_add(o3v, dqv[:, :, 0:5:2], dqv[:, :, 1:6:2])
                nc.vector.tensor_mul(out3[:], out3[:], pxy3[:, 24 * q0: 24 * q0 + 3 * n8])
                n1s = wk.tile([SC, n8], F32, tag="n1")
                nc.vector.tensor_reduce(n1s[:], o3v, axis=AX.X, op=ALU.add)
                rs = wk.tile([SC, n8], F32, tag="rs")
                nc.vector.reciprocal(rs[:], dq[:, 6:W:8])
                nc.vector.tensor_mul(n1s[:], n1s[:], rs[:])
                nc.vector.tensor_reduce(dmacc[:, j:j + 1], n1s[:], axis=AX.X, op=ALU.max)

            done_pairs = set()
            for step in range(NP2 + 2):
                if step < NP2:
                    em2a(step)
                if 2 <= step:
                    em2b(step - 2)
                    done_pairs.add(step - 2)
                for j, (q0, nqj, p0, p1) in enumerate(CHK[:-1]):
                    if p1 == step - 3 and p1 in done_pairs and j in dTp:
                        phaseC(j)
            phaseC(len(CHK) - 1)
            bmax = wk.tile([SC, 1], F32)
            nc.vector.tensor_reduce(bmax[:], dmacc[:], axis=AX.X, op=ALU.max)
            nc.vector.tensor_scalar_max(bmax[:], bmax[:], 0.0)

            # ---- per-sample scores -> w ----
            args = wk.tile([SC, 1], F32)
            nc.vector.tensor_scalar(args[:], avg[:], float(BETA_SPEED / P), 0.0, op0=ALU.mult, op1=ALU.add)
            nc.vector.tensor_add(args[:], args[:], worst[:])
            ca_pen = wk.tile([SC, 1], F32)
            nc.vector.tensor_scalar(ca_pen[:], camax[:], float(MAX_CA), 0.0, op0=ALU.subtract, op1=ALU.max)
            nc.vector.tensor_sub(args[:], args[:], ca_pen[:])
            w = wk.tile([SC, 1], F32)
            if fuse_w:
                # boundary clamp provably inactive: one fused exp
                nc.vector.tensor_sub(args[:], args[:], bmax[:])
                nc.scalar.activation(w[:], args[:], AF.Exp)
            else:
                e1 = wk.tile([SC, 1], F32)
                nc.scalar.activation(e1[:], args[:], AF.Exp)
                e2 = wk.tile([SC, 1], F32)
                nc.scalar.activation(e2[:], bmax[:], AF.Exp, scale=-1.0)
                nc.vector.tensor_scalar_max(e2[:], e2[:], 1e-32)
                nc.vector.tensor_mul(w[:], e1[:], e2[:])

            # ---- partial sums ----
            op17 = pt2.tile([17, 1], F32, tag="t2")
            nc.tensor.matmul(op17[:], cf[:], w[:], start=True, stop=True)
            o17 = wk.tile([17, 1], F32)
            nc.vector.tensor_copy(o17[:], op17[:])
            nc.sync.dma_start(d_out, o17[:])

    nc.compile()
    return nc


def _ref_replay(curves, dT, xs, ys, M, Md, M2d, dfuns):
    """fp64 replay of the reference pipeline; dfuns gives per-boundary
    max-signed-distance evaluators. Returns the [8,2] weighted curve."""
    D1 = _diff_mat(7)
    D1b = _diff_mat(6)[:, :7]
    pts = np.einsum('pk,skd->spd', M, curves)
    v_t = np.einsum('pk,skd->spd', (7.0 / dT) * (Md @ D1), curves)
    a_t = np.einsum('pk,skd->spd', (42.0 / (dT * dT)) * (M2d @ D1b @ D1), curves)
    speeds = np.linalg.norm(v_t, axis=2)
    ut = v_t / speeds[:, :, None]
    avg = speeds.mean(1)
    lin = (a_t * ut).sum(2)
    blim = np.interp(speeds.reshape(-1), xs, ys).reshape(speeds.shape)
    worst = np.minimum(lin - blim, 0.0).min(1)
    ca2 = (a_t * a_t).sum(2) - lin * lin
    camax = np.sqrt(np.maximum(ca2, 0.0).max(1))
    ca_pen = np.maximum(camax - MAX_CA, 0.0)
    pen = np.maximum(np.maximum(dfuns[0](pts), dfuns[1](pts)), 0.0)
    logw = BETA_SPEED * avg + worst - ca_pen - pen
    logw -= logw.max()
    w = np.exp(logw)
    w = np.maximum(w, 1e-300)
    return (w[:, None, None] * curves).sum(0) / w.sum()


def _mk_dfun(bpts, bnrm):
    b = np.ascontiguousarray(bpts, np.float64)
    n = np.ascontiguousarray(bnrm, np.float64)
    b2 = (b * b).sum(1)

    def dfun(pts):
        S = pts.shape[0]
        out = np.empty(S)
        for lo in range(0, S, 64):
            q = pts[lo:lo + 64]
            sc = 2.0 * (q @ b.T)
            sc -= b2[None, None, :]
            idx = sc.argmax(-1)
            cb = b[idx]
            cn = n[idx]
            out[lo:lo + 64] = ((cb - q) * cn).sum(-1).max(-1)
        return out
    return dfun


def _mk_dfun_soft(bpts, bnrm, Csh, K, a, c0, theta=None):
    """Simulates the device soft-select: H = exp(K(s1 - mb) - 25) with bf16
    underflow flush (or H = 1[mb - s1 <= theta] when theta is given);
    d = (sel_e - px nx - py ny)/count."""
    b = np.ascontiguousarray(bpts, np.float64)
    n = np.ascontiguousarray(bnrm, np.float64)
    b2C = (b * b).sum(1) + Csh
    e = (b * n).sum(1)

    def dfun(pts):
        S = pts.shape[0]
        out = np.empty(S)
        for lo in range(0, S, 64):
            q = pts[lo:lo + 64]                       # [s,P,2]
            s1 = 2.0 * (q @ b.T) - b2C[None, None, :]
            mb = a * np.linalg.norm(q, axis=-1) - c0  # [s,P]
            if theta is None:
                H = np.exp(np.maximum(K * (s1 - mb[:, :, None]) - 25.0, -700))
                H[H < 1.2e-38] = 0.0
            else:
                H = (mb[:, :, None] - s1 <= theta).astype(np.float64)
            cnt = H.sum(-1)
            se = H @ e
            sx = H @ n[:, 0]
            sy = H @ n[:, 1]
            with np.errstate(divide='ignore', invalid='ignore'):
                d = (se - q[:, :, 0] * sx - q[:, :, 1] * sy) / cnt
            d[~np.isfinite(d)] = 1e30   # underflowed column -> force gate failure
            out[lo:lo + 64] = d.max(-1)
        return out
    return dfun


def _host_prep(curve, noise, deltaT, speeds_x, braking_y, bezierM, bezierMd, bezierM2d,
               inner_boundary, inner_normals, outer_boundary, outer_normals):
    f64 = np.float64
    dT = float(deltaT)
    curves = (curve[None].astype(f64) + noise.astype(f64))  # [1024, 8, 2]

    M = bezierM.astype(f64)
    Md = bezierMd.astype(f64)
    M2d = bezierM2d.astype(f64)
    D1 = _diff_mat(7)
    D1b = _diff_mat(6)[:, :7]
    R = np.zeros((8, 180), f64)
    R[:, 0:60] = M.T
    R[:, 60:120] = (7.0 / dT) * (Md @ D1).T
    R[:, 120:180] = (42.0 / (dT * dT)) * (M2d @ D1b @ D1).T

    bset = [(inner_boundary.astype(f64), inner_normals.astype(f64)),
            (outer_boundary.astype(f64), outer_normals.astype(f64))]
    xs = speeds_x.astype(f64)
    ys = braking_y.astype(f64)

    cmax = max(float(np.abs(curves).max()), 1.0)
    Csh = 2.0 * cmax * cmax + 1.0
    pts_all = np.einsum('pk,skd->spd', M, curves)

    ref_full = _ref_replay(curves, dT, xs, ys, M, Md, M2d,
                           [_mk_dfun(*bset[0]), _mk_dfun(*bset[1])])

    # ---- adaptive subsampling + device-math validation ----
    nch = None
    for try_nch in (1, 2, 4, 8, 16):
        cap = 64 * try_nch
        subs = []
        for (b, n) in bset:
            nb = b.shape[0]
            if nb <= cap:
                idx = np.arange(nb)
            else:
                idx = np.unique(np.round(np.linspace(0, nb - 1, cap)).astype(int))
            subs.append(idx)
        # mb-bound constants and gap for K
        mbc = []
        gap = 0.05
        smax = 1.0
        for bd in range(2):
            b = bset[bd][0][subs[bd]]
            b2C = (b * b).sum(1) + Csh
            a = 2.0 * float(np.sqrt((b * b).sum(1).max()))
            c0 = float(b2C.min())
            mbc.append((a, c0))
            s1 = 2.0 * (pts_all.reshape(-1, 2) @ b.T) - b2C[None, :]
            mbq = a * np.linalg.norm(pts_all.reshape(-1, 2), axis=-1) - c0
            gap = max(gap, float((mbq - s1.max(1)).max()))
            smax = max(smax, float(np.abs(s1).max()), float(np.abs(mbq).max()))
        # +0.5 margin covers fp16 point/mb rounding between host and device
        K = float(min(2.0 ** 17 / smax, 55.0 / (gap + 0.5)))
        dfs = [_mk_dfun_soft(bset[bd][0][subs[bd]], bset[bd][1][subs[bd]],
                             Csh, K, mbc[bd][0], mbc[bd][1]) for bd in range(2)]
        out_s = _ref_replay(curves, dT, xs, ys, M, Md, M2d, dfs)
        err = np.abs(out_s - ref_full).max() / (np.abs(ref_full).max() + 1e-12)
        if err < 1e-3 or try_nch == 16:
            nch = try_nch
            break

    # ---- Newton-sqrt ranges (table-free sqrt on the Vector engine),
    # boundary-clamp activity, both host-verified ----
    D1 = _diff_mat(7)
    D1b = _diff_mat(6)[:, :7]
    v_t = np.einsum('pk,skd->spd', (7.0 / dT) * (Md @ D1), curves)
    a_t = np.einsum('pk,skd->spd', (42.0 / (dT * dT)) * (M2d @ D1b @ D1), curves)
    spd2v = (v_t * v_t).sum(-1)
    ut = v_t / np.sqrt(spd2v)[:, :, None]
    linv = (a_t * ut).sum(2)
    ca2v = np.maximum((a_t * a_t).sum(2) - linv * linv, 0.0)

    def newton_ok(m, Mx, iters=3, tol=1e-4):
        if not (m > 0 and m > 1e-9 * Mx):
            return False
        x = np.geomspace(m, Mx, 2000)
        G = np.sqrt(m * Mx)
        y = (x + G) / (np.sqrt(m) + np.sqrt(Mx))
        for _ in range(iters):
            y = 0.5 * (y + x / y)
        return bool(np.abs(y / np.sqrt(x) - 1.0).max() < tol)

    def amb_ok(tol=1e-4):
        avx = np.abs(v_t[..., 0])
        avy = np.abs(v_t[..., 1])
        y = np.maximum(avx, avy) + 0.5 * np.minimum(avx, avy)
        if not (y > 0).all():
            return False
        for _ in range(2):
            y = 0.5 * (y + spd2v / y)
        return bool(np.abs(y / np.sqrt(spd2v) - 1.0).max() < tol)

    lo2 = (0.8 * MAX_CA) ** 2
    rng_c = (lo2, float(max(ca2v.max() * 1.1, 2.0 * lo2)))
    if amb_ok() and newton_ok(*rng_c):
        sq = ("amb", rng_c)
    elif newton_ok(*rng_c):
        sq = ("hybrid", rng_c)
    else:
        sq = ("act",)
    pen = np.maximum(np.maximum(dfs[0](pts_all), dfs[1](pts_all)), 0.0)
    fuse_w = bool(pen.max() < 60.0)

    # is_le split of the H computation (half on DVE) measured slower than
    # exp-everywhere on hardware (DVE becomes co-pacer and the wide
    # acceptance band costs accuracy) -- keep disabled
    noise_est = 0.55 * max(smax / 3700.0, 0.05)
    theta = float(gap + 0.5 + noise_est + 3.0 / K)
    split_h = False

    # ---- boundary tables: chunk c rows 0-63 = bd0[64c:..], 64-127 = bd1 ----
    def bf16_rne(x):
        x32 = np.asarray(x, np.float32)
        u = x32.view(np.uint32)
        r = ((u + 0x7FFF + ((u >> 16) & 1)) & 0xFFFF0000).astype(np.uint32)
        return r.view(np.float32).astype(f64)

    NH = 64 * nch
    bG = np.zeros((8, 128 * nch), f64)
    tb_sb = np.zeros((128, 16 * nch), f64)
    for bd in range(2):
        idx = subs[bd]
        nb = len(idx)
        b = np.zeros((NH, 2), f64)
        n = np.zeros((NH, 2), f64)
        b[:nb] = bset[bd][0][idx]
        n[:nb] = bset[bd][1][idx]
        b2C = (b * b).sum(1) + Csh
        b2C[nb:] = 60000.0
        e = (b * n).sum(1)
        for c in range(nch):
            sl = slice(c * 64, (c + 1) * 64)
            col = slice(c * 128 + bd * 64, c * 128 + bd * 64 + 64)
            bG[0, col] = -2 * b[sl, 0]
            bG[1, col] = -2 * b[sl, 1]
            hi = np.float16(b2C[sl]).astype(f64)
            bG[2, col] = hi
            bG[3, col] = b2C[sl] - hi
            bG[4 + 2 * bd, col] = 1.0
            bG[5 + 2 * bd, col] = 1.0
            # select table: within-chunk row = bd*64 + j
            t8 = np.zeros((64, 8), f64)
            for v, vals in enumerate((e[sl], n[sl, 0], n[sl, 1])):
                hi = bf16_rne(vals)
                t8[:, 2 * v] = hi
                t8[:, 2 * v + 1] = bf16_rne(vals - hi)
            t8[:, 6] = (np.arange(c * 64, (c + 1) * 64) < nb).astype(f64)
            tb_sb[bd * 64:(bd + 1) * 64, c * 16 + bd * 8: c * 16 + bd * 8 + 8] = t8

    # interp constants (+ linearity detection vs the endpoint line)
    dx = np.diff(xs)
    dx_safe = np.where(dx > 0, dx, 1.0)
    m = np.where(dx > 0, np.diff(ys) / dx_safe, 0.0)
    lin = False
    if np.all(dx > 0) and xs[-1] > xs[0]:
        m0 = (ys[-1] - ys[0]) / (xs[-1] - xs[0])
        dev = np.abs(ys - (ys[0] + m0 * (xs - xs[0]))).max()
        if dev < 1e-4:
            lin = True
            m = np.full_like(m, m0)
    interp = (lin, xs, dx_safe, m, float(ys[0]),
              float(min(ys[0], ys[-1])), float(max(ys[0], ys[-1])))

    import ml_dtypes
    tb_bf16 = tb_sb.astype(ml_dtypes.bfloat16)
    ins = []
    for c in range(NCORES):
        cs = curves[c * SC:(c + 1) * SC]  # [128, 8, 2]
        cv = np.ascontiguousarray(cs.transpose(2, 1, 0).reshape(16, SC)).astype(np.float32)
        cf17 = np.concatenate([cs.reshape(SC, 16), np.ones((SC, 1))], 1).astype(np.float32)
        # pg grid [8, P*SC], col = p*SC + s: [px;py;1;1;mb0hi;mb0lo;mb1hi;mb1lo]
        pts = pts_all[c * SC:(c + 1) * SC]            # [128, 60, 2]
        pnorm = np.linalg.norm(pts, axis=-1)          # [128, 60]
        pgt = np.ones((8, P * SC), f64)
        pgt[0] = pts[:, :, 0].T.ravel()
        pgt[1] = pts[:, :, 1].T.ravel()
        for bd in range(2):
            a, c0 = mbc[bd]
            mb = (a * pnorm - c0).T.ravel()
            hi = np.float16(mb).astype(f64)
            pgt[4 + 2 * bd] = hi
            pgt[5 + 2 * bd] = mb - hi
        ins.append(dict(
            cv=cv, cf17=cf17, pgt=pgt.astype(np.float16),
            Rm=R.astype(np.float32), bG=bG.astype(np.float16),
            tb=tb_bf16,
            Kv=np.full((SC, 1), -K, np.float32),
            Th=np.full((SC, 1), theta, np.float32),
        ))
    return ins, (interp, nch, sq, fuse_w, K, split_h)


def kernel(curve, noise, deltaT, speeds_x, braking_y, bezierM, bezierMd, bezierM2d,
           inner_boundary, inner_normals, outer_boundary, outer_normals):
    in_maps, (interp, nch, sq, fuse_w, K, split_h) = _host_prep(
        curve, noise, deltaT, speeds_x, braking_y, bezierM, bezierMd, bezierM2d,
        inner_boundary, inner_normals, outer_boundary, outer_normals)

    key = (interp[0], tuple(np.round(interp[1], 9)), tuple(np.round(interp[3], 9)),
           round(interp[4], 9), nch, fuse_w, split_h,
           tuple(np.round(np.array([x for t in sq[1:] for x in t]), 6)) if sq[0] == "newton" else sq)
    if key not in _cache:
        _cache.clear()
        _cache[key] = _build_program(interp, nch, sq, fuse_w, split_h)
    nc = _cache[key]

    res = bass_utils.run_bass_kernel_spmd(nc, in_maps, core_ids=list(range(NCORES)))
    outs = res.results
    num = np.zeros(16, np.float64)
    Z = 0.0
    for c in range(NCORES):
        o = np.asarray(outs[c]["out17"]).reshape(17)
        num += o[:16].astype(np.float64)
        Z += float(o[16])
    return (num / Z).reshape(8, 2).astype(np.float32)


if __name__ == "__main__":
    import reference
    inp = {k: np.asarray(v) for k, v in reference.setup_inputs().items()}
    out = kernel(**inp)
    exp = np.asarray(reference.reference(**reference.setup_inputs()))
    err = np.abs(out - exp).max() / (np.abs(exp).max() + 1e-12)
    print("Relative error:", err)
